# revision 9
# baseline (speedup 1.0000x reference)
"""Trainium2 Bass kernel for nn_AvgPoolVectorsPerWSI (segment-mean over groups).

Math: x [N=2048, M=512, 7, 7], idx [N] in [0,64)
  out[g, m] = mean over {n: idx[n]==g} and spatial of x[n, m, :, :]  -> [64, 512, 1, 1]

Strategy (no collectives needed):
  - Shard over M: core k handles an m-slice of 64 channels. Each core reads
    its x slice [2048, 64, 49] (25.7 MB) once. The 16-engine DMA stream runs
    at fabric line rate (~430 GB/s, 466 ns per 12.5 KB row descriptor), so
    the stream time (~62 us) is the floor; everything else hides behind it.
  - Per n-tile of 128 samples: DVE spatially j-reduces ALL 64 channels
    ([128, 64, 49] -> [128, 64], ~3.4 us < 3.87 us/tile DMA pace), then PE
    does one tiny fp32 matmul psum[g, m] += w[n, g]^T @ xs[n, m] into a
    single PSUM bank, with w the scale-weighted one-hot (scale =
    1/(count_g*49)) generated ON DEVICE from a 74 KB aux tensor.
  - Tail minimization (the stream end -> block end path is what matters;
    the ~7 us NRT postamble after it is fixed):
      * tile 15 is DMA'd as 4 column pieces so the last exposed DVE reduce
        is only 16 channels (~0.9 us);
      * the per-tile matmul is split into m[0:48] / m[48:64] halves so only
        a 16-channel matmul sliver depends on the last piece;
      * the PSUM->SBUF copies run on the Scalar(ACT) engine (closer to
        PSUM), keeping DVE off the critical path;
      * aux is loaded via the ACT HWDGE ring in parallel with the x stream;
      * the final out DMA has no completion wait -- NRT's postamble
        (sem resets, then dma_rearm) drains the ring with ~5 us of margin.

Raw Block implementation (not Tile): the walrus matmul/DMA lowerings only
accept ONE attached sync-wait per instruction; standalone wait_ge
instructions sidestep that.

DMA-completion semaphores: slot s's sem counts cumulatively (tile s: +16,
tile s+8: +16 more; slot 7 additionally gets tile 15's 4 pieces at +16
each). Slot reuse is ordered by red_sem (the DMA for tile t+8 waits for
tile t's reduce), so a slot's sem is only re-used strictly after its
previous tile was consumed.
"""

from contextlib import ExitStack

import numpy as np

import concourse.bass as bass
import concourse.mybir as mybir
from concourse.bass_utils import run_bass_kernel_spmd

N = 2048          # samples
M = 512           # channels
HW = 49           # spatial (7*7)
G = 64            # groups
CORES = 8
ML = M // CORES   # 64 channels per core
F = ML * HW       # 3136 floats per (n, core)
P = 128           # partitions per tile
NT = N // P       # 16 n-tiles
BUFS = 8          # x-tile buffer depth == number of DMA semaphores
NP15 = 4          # tile 15 arrives in 4 column pieces
PC = ML // NP15   # 16 channels per piece
PF = PC * HW      # 784 columns per piece
MA = 48           # matmul half A covers m[0:48]; half B covers m[48:64]

F32 = mybir.dt.float32


def _build():
    nc = bass.Bass(trn_type="TRN2", target_bir_lowering=False)
    x_ext = nc.declare_dram_parameter("x", [N, F], F32, isOutput=False)
    # aux[:, 0:64] iota row, aux[:, 64:128] scale row, aux[:, 128:144] idx
    aux_ext = nc.declare_dram_parameter("aux", [P, G + G + NT], F32,
                                        isOutput=False)
    out_ext = nc.declare_dram_parameter("out", [G, ML], F32, isOutput=True)

    x_t = x_ext.ap().rearrange("(t p) f -> t p f", p=P)  # [16, 128, 3136]

    with ExitStack() as ctx:
        x_buf = ctx.enter_context(nc.sbuf_tensor([P, BUFS * F], F32))
        xs_buf = ctx.enter_context(nc.sbuf_tensor([P, BUFS * ML], F32))
        aux_sb = ctx.enter_context(nc.sbuf_tensor([P, G + G + NT], F32))
        w_sb = ctx.enter_context(nc.sbuf_tensor([P, NT * G], F32))
        out_sb = ctx.enter_context(nc.sbuf_tensor([G, ML], F32))
        warm_sb = ctx.enter_context(nc.sbuf_tensor([P, 1], F32))
        # separate tensors -> separate PSUM banks: a matmul with start=True
        # clears the has_written bits of its WHOLE bank, so the two
        # interleaved accumulation halves must not share one
        psum_a = ctx.enter_context(nc.psum_tensor([G, MA], F32))
        psum_b = ctx.enter_context(nc.psum_tensor([G, ML - MA], F32))
        dma_x = [
            ctx.enter_context(nc.semaphore(name=f"dma_x{s}"))
            for s in range(BUFS)
        ]
        dma_a = ctx.enter_context(nc.semaphore())   # +16 when aux resident
        dma_o = ctx.enter_context(nc.semaphore())   # +16 when out written (never waited)
        wg_sem = ctx.enter_context(nc.semaphore())  # +1 when w generated
        red_sem = ctx.enter_context(nc.semaphore())  # +1 per reduce op
        pe_sem = ctx.enter_context(nc.semaphore())   # +1 per matmul half
        fin_sem = ctx.enter_context(nc.semaphore())  # +1 per out_sb copy
        block = ctx.enter_context(nc.Block())

        # dma_x thresholds: slot s<7 -> 16 (tile s), 32 (tile s+8);
        # slot 7 -> 16 (tile 7), then 32/48/64/80 (tile 15 pieces)
        def xthresh(t):
            return 16 + 16 * (t // BUFS)

        # ---- x-stream DMA program (SP / HWDGE ring, FIFO) ----
        @block.sync
        def _(sync):
            # aux first: it is tiny (74 KB) and w-gen + the whole DVE chain
            # hang off it -- on the ACT ring it lands too late and the DVE
            # start deficit compounds into a pipeline stall
            sync.dma_start(out=aux_sb[:, :], in_=aux_ext.ap()).then_inc(
                dma_a, 16
            )
            for t in range(NT - 1):
                if t >= BUFS:
                    # slot reuse: tile t-8's reduce has consumed the slot
                    sync.wait_ge(red_sem, t - BUFS + 1)
                slot = t % BUFS
                sync.dma_start(
                    out=x_buf[:, slot * F:(slot + 1) * F], in_=x_t[t]
                ).then_inc(dma_x[slot], 16)
            # tile 15 -> slot 7, in 4 column pieces
            sync.wait_ge(red_sem, 8)
            for p in range(NP15):
                sync.dma_start(
                    out=x_buf[:, 7 * F + p * PF:7 * F + (p + 1) * PF],
                    in_=x_t[NT - 1][:, p * PF:(p + 1) * PF],
                ).then_inc(dma_x[7], 16)
            # out: issue and do NOT wait -- NRT's postamble drains the ring
            # (the inc is required by walrus's DGE lowering but never waited)
            sync.wait_ge(fin_sem, 2)
            sync.dma_start(out=out_ext.ap(), in_=out_sb[:, :]).then_inc(
                dma_o, 16
            )

        # ---- VectorE: w generation, per-tile spatial j-reduction ----
        @block.vector
        def _(vector):
            # generate the scale-weighted one-hot from idx:
            #   w[p, t*G+g] = (idx[t*128+p] == g) * scale[g]
            vector.wait_ge(dma_a, 16)
            for t in range(NT):
                wg = vector.scalar_tensor_tensor(
                    out=w_sb[:, t * G:(t + 1) * G],
                    in0=aux_sb[:, 0:G],
                    scalar=aux_sb[:, 2 * G + t:2 * G + t + 1],
                    in1=aux_sb[:, G:2 * G],
                    op0=mybir.AluOpType.is_equal,
                    op1=mybir.AluOpType.mult,
                )
            wg.then_inc(wg_sem, 1)

            for t in range(NT - 1):
                if t >= BUFS:
                    # xs slot reuse: tile t-8's matmul half B is done
                    vector.wait_ge(pe_sem, 2 * (t - BUFS) + 2)
                vector.wait_ge(dma_x[t % BUFS], xthresh(t))
                slot = t % BUFS
                vector.tensor_reduce(
                    out=xs_buf[:, slot * ML:(slot + 1) * ML],
                    in_=x_buf[:, slot * F:(slot + 1) * F].rearrange(
                        "p (m j) -> p m j", j=HW
                    ),
                    axis=mybir.AxisListType.X,
                    op=mybir.AluOpType.add,
                ).then_inc(red_sem, 1)
            # tile 15: 4 piece reduces (16 channels each)
            vector.wait_ge(pe_sem, 2 * (NT - 1 - BUFS) + 2)
            for p in range(NP15):
                vector.wait_ge(dma_x[7], 32 + 16 * p)
                vector.tensor_reduce(
                    out=xs_buf[:, 7 * ML + p * PC:7 * ML + (p + 1) * PC],
                    in_=x_buf[:, 7 * F + p * PF:7 * F + (p + 1) * PF]
                    .rearrange("p (m j) -> p m j", j=HW),
                    axis=mybir.AxisListType.X,
                    op=mybir.AluOpType.add,
                ).then_inc(red_sem, 1)

        # ---- TensorE: segment-sum accumulation (fp32, one PSUM bank) ----
        @block.tensor
        def _(tensor):
            tensor.wait_ge(wg_sem, 1)
            for t in range(NT - 1):
                tensor.wait_ge(red_sem, t + 1)
                slot = t % BUFS
                wt = w_sb[:, t * G:(t + 1) * G]
                tensor.matmul(
                    out=psum_a[:, :],
                    lhsT=wt,
                    rhs=xs_buf[:, slot * ML:slot * ML + MA],
                    start=(t == 0),
                    stop=False,
                ).then_inc(pe_sem, 1)
                tensor.matmul(
                    out=psum_b[:, :],
                    lhsT=wt,
                    rhs=xs_buf[:, slot * ML + MA:slot * ML + ML],
                    start=(t == 0),
                    stop=False,
                ).then_inc(pe_sem, 1)
            # tile 15: half A after pieces 0-2, half B after piece 3
            w15 = w_sb[:, (NT - 1) * G:NT * G]
            tensor.wait_ge(red_sem, (NT - 1) + 3)
            tensor.matmul(
                out=psum_a[:, :],
                lhsT=w15,
                rhs=xs_buf[:, 7 * ML:7 * ML + MA],
                start=False,
                stop=True,
            ).then_inc(pe_sem, 1)
            tensor.wait_ge(red_sem, (NT - 1) + 4)
            tensor.matmul(
                out=psum_b[:, :],
                lhsT=w15,
                rhs=xs_buf[:, 7 * ML + MA:7 * ML + ML],
                start=False,
                stop=True,
            ).then_inc(pe_sem, 1)

        # ---- ScalarE (ACT): PSUM -> SBUF copies ----
        @block.scalar
        def _(scalar):
            # warmup: the first activation op lazily DMAs the ACT function
            # table (~1.3 us); trigger it here so the tail copies are cheap
            scalar.activation(
                out=warm_sb[:, :],
                in_=warm_sb[:, :],
                func=mybir.ActivationFunctionType.Copy,
            )
            scalar.wait_ge(pe_sem, 2 * NT - 1)
            scalar.activation(
                out=out_sb[:, 0:MA],
                in_=psum_a[:, :],
                func=mybir.ActivationFunctionType.Copy,
            ).then_inc(fin_sem, 1)
            scalar.wait_ge(pe_sem, 2 * NT)
            scalar.activation(
                out=out_sb[:, MA:ML],
                in_=psum_b[:, :],
                func=mybir.ActivationFunctionType.Copy,
            ).then_inc(fin_sem, 1)

    return nc


def _prepare(x, idx):
    x = np.asarray(x)
    if x.dtype != np.float32:
        x = x.astype(np.float32)
    idx = np.asarray(idx).astype(np.int64)
    counts = np.bincount(idx, minlength=G).astype(np.float64)
    scale = np.where(counts > 0, 1.0 / (counts * HW), 0.0).astype(np.float32)
    aux = np.zeros((P, G + G + NT), np.float32)
    aux[:, 0:G] = np.arange(G, dtype=np.float32)[None, :]
    aux[:, G:2 * G] = scale[None, :]
    aux[:, 2 * G:] = idx.reshape(NT, P).T.astype(np.float32)
    xr = x.reshape(N, M, HW)
    in_maps = []
    for k in range(CORES):
        shard = np.ascontiguousarray(xr[:, k * ML:(k + 1) * ML, :]).reshape(N, F)
        in_maps.append({"x": shard, "aux": aux})
    return in_maps


def run(x, tensor_list_assignmentindices, trace=False):
    in_maps = _prepare(x, tensor_list_assignmentindices)
    nc = _build()
    res = run_bass_kernel_spmd(nc, in_maps, core_ids=list(range(CORES)), trace=trace)
    outs = [np.asarray(r["out"]) for r in res.results]
    out = np.concatenate(outs, axis=1)  # [G, M]
    return out.reshape(G, M, 1, 1).astype(np.float32), res.exec_time_ns


def kernel(**inputs):
    out, _ = run(inputs["x"], inputs["tensor_list_assignmentindices"], trace=False)
    return out


# revision 11
# speedup vs baseline: 1.1762x; 1.1762x over previous
"""Trainium2 Bass kernel for nn_AvgPoolVectorsPerWSI (segment-mean over groups).

Math: x [N=2048, M=512, 7, 7], idx [N] in [0,64)
  out[g, m] = mean over {n: idx[n]==g} and spatial of x[n, m, :, :]  -> [64, 512, 1, 1]

Strategy (no collectives needed):
  - Shard over M: core k handles an m-slice of 64 channels. Each core
    streams its x slice [2048, 64, 49] (25.7 MB) once at fabric line rate
    (~430 GB/s, 466 ns per 12.5 KB row descriptor); the ~62 us stream is
    the floor and everything else hides behind it. 16 n-tiles of 128 rows
    (128-partition DMAs only -- partition-partial bulk DMAs mis-lower).
  - Per tile: PE accumulates the first MC=10 channels raw
    (psum_raw[g, (m,j)] += w^T @ x, one 490-col fp32 chunk), DVE spatially
    j-reduces the other 54 channels ([128, 54, 49] -> [128, 54],
    ~2.9 us/tile vs ~3.9 us/tile DMA pace -- real headroom), then PE adds
    two small fp32 matmuls psum[g, m] += w^T @ xs, split m[10:46]/m[46:64]
    so only a sliver depends on the last DMA piece. w is the
    scale-weighted one-hot (scale = 1/(count_g*49)) generated ON DEVICE
    from a 74 KB aux tensor loaded first on the ring (w-gen and the whole
    DVE chain hang off it).
  - Tail minimization (stream end -> block end is what matters; the ~7 us
    NRT postamble after it is fixed):
      * tile 15 arrives in 4 column pieces (raw chunk, then 3x18 channels)
        so the last exposed reduce is ~1 us;
      * PSUM->SBUF copies run on ScalarE, warmed up at start (the first
        activation op lazily loads the ACT table, ~1.3 us);
      * DVE j-reduces psum_raw into out[:, 0:10] concurrently with the
        final matmul sliver;
      * the final out DMA has no completion wait -- NRT's postamble
        (sem resets, then dma_rearm) drains the ring with ~5 us margin.

Raw Block implementation (not Tile): the walrus matmul/DMA lowerings only
accept ONE attached sync-wait per instruction; standalone wait_ge
instructions sidestep that.

PSUM: a matmul with start=True clears the has_written bits of its WHOLE
bank, so the three accumulation streams (raw / half A / half B) live in
three separate psum tensors.

Known environmental hazard: SDMA engine 15 episodically runs ~20-25% slow,
and every tile's completion semaphore needs all 16 engines, so on such
runs the stream is engine-15-paced (~+14 us) regardless of kernel
structure; partition-exclusion workarounds mis-lower (see above), so this
is accepted as run-to-run variance.
"""

from contextlib import ExitStack

import numpy as np

import concourse.bass as bass
import concourse.mybir as mybir
from concourse.bass_utils import run_bass_kernel_spmd

N = 2048          # samples
M = 512           # channels
HW = 49           # spatial (7*7)
G = 64            # groups
CORES = 8
ML = M // CORES   # 64 channels per core
F = ML * HW       # 3136 floats per (n, core)
P = 128           # partitions per tile
NT = N // P       # 16 n-tiles
BUFS = 8          # x slot ring depth == number of DMA semaphores
MC = 10           # channels on the PE raw path
FC = MC * HW      # 490 raw columns (single PSUM bank)
MV = ML - MC      # 54 channels on the DVE reduce path
# tile 15 pieces: raw chunk [0:FC], then channel blocks [10:28),[28:46),[46:64)
PIECE_CH = [(MC, 28), (28, 46), (46, 64)]
MB = 46           # matmul half A covers m[MC:46]; half B covers m[46:64]

F32 = mybir.dt.float32


def _build():
    nc = bass.Bass(trn_type="TRN2", target_bir_lowering=False)
    x_ext = nc.declare_dram_parameter("x", [N, F], F32, isOutput=False)
    # aux[:, 0:64] iota row, aux[:, 64:128] scale row, aux[:, 128:144] idx
    aux_ext = nc.declare_dram_parameter("aux", [P, G + G + NT], F32,
                                        isOutput=False)
    out_ext = nc.declare_dram_parameter("out", [G, ML], F32, isOutput=True)

    x_t = x_ext.ap().rearrange("(t p) f -> t p f", p=P)  # [16, 128, 3136]

    with ExitStack() as ctx:
        x_buf = ctx.enter_context(nc.sbuf_tensor([P, BUFS * F], F32))
        xs_buf = ctx.enter_context(nc.sbuf_tensor([P, BUFS * MV], F32))
        aux_sb = ctx.enter_context(nc.sbuf_tensor([P, G + G + NT], F32))
        w_sb = ctx.enter_context(nc.sbuf_tensor([P, NT * G], F32))
        out_sb = ctx.enter_context(nc.sbuf_tensor([G, ML], F32))
        warm_sb = ctx.enter_context(nc.sbuf_tensor([P, 1], F32))
        # separate tensors -> separate PSUM accumulation state
        psum_raw = ctx.enter_context(nc.psum_tensor([G, FC], F32))
        psum_a = ctx.enter_context(nc.psum_tensor([G, MB - MC], F32))
        psum_b = ctx.enter_context(nc.psum_tensor([G, ML - MB], F32))
        dma_x = [
            ctx.enter_context(nc.semaphore(name=f"dma_x{s}"))
            for s in range(BUFS)
        ]
        dma_a = ctx.enter_context(nc.semaphore())   # +16 when aux resident
        dma_o = ctx.enter_context(nc.semaphore())   # out DMA (never waited)
        wg_sem = ctx.enter_context(nc.semaphore())  # +1 when w generated
        red_sem = ctx.enter_context(nc.semaphore())  # +1 per DVE reduce op
        pe_sem = ctx.enter_context(nc.semaphore())   # +1 per PE matmul
        fin_sem = ctx.enter_context(nc.semaphore())  # +1 per out_sb writer
        block = ctx.enter_context(nc.Block())

        # cumulative dma_x thresholds: slot s serves tiles s and s+8;
        # tile 15 (slot 7) arrives as 4 pieces of +16 each (32/48/64/80)
        def xthresh(t):
            return 16 * (t // BUFS) + 16

        # pe_sem incs: 3 per tile (raw, mm_a, mm_b)
        def pe_count(t):
            return 3 * (t + 1)

        # ---- x-stream DMA program (SP / HWDGE ring, FIFO) ----
        @block.sync
        def _(sync):
            # aux first: w-gen and the whole DVE chain hang off it
            sync.dma_start(out=aux_sb[:, :], in_=aux_ext.ap()).then_inc(
                dma_a, 16
            )
            for t in range(NT - 1):
                if t >= BUFS:
                    # slot reuse: all three matmuls of tile t-8 are done
                    # (mm_b implies the DVE reduce consumed the slot too)
                    sync.wait_ge(pe_sem, pe_count(t - BUFS))
                slot = t % BUFS
                sync.dma_start(
                    out=x_buf[:, slot * F:(slot + 1) * F], in_=x_t[t]
                ).then_inc(dma_x[slot], 16)
            # tile 15 -> slot 7, 4 column pieces
            sync.wait_ge(pe_sem, pe_count(NT - 1 - BUFS))
            bounds = [0, FC] + [hi * HW for _, hi in PIECE_CH]
            for p in range(4):
                lo, hi = bounds[p], bounds[p + 1]
                sync.dma_start(
                    out=x_buf[:, 7 * F + lo:7 * F + hi],
                    in_=x_t[NT - 1][:, lo:hi],
                ).then_inc(dma_x[7], 16)
            # out: issue and do NOT wait -- NRT's postamble drains the ring
            sync.wait_ge(fin_sem, 3)
            sync.dma_start(out=out_ext.ap(), in_=out_sb[:, :]).then_inc(
                dma_o, 16
            )

        # ---- VectorE: w generation, spatial j-reduction, raw epilogue ----
        @block.vector
        def _(vector):
            # w[p, t*G+g] = (idx[t*128+p] == g) * scale[g]
            vector.wait_ge(dma_a, 16)
            for t in range(NT):
                wg = vector.scalar_tensor_tensor(
                    out=w_sb[:, t * G:(t + 1) * G],
                    in0=aux_sb[:, 0:G],
                    scalar=aux_sb[:, 2 * G + t:2 * G + t + 1],
                    in1=aux_sb[:, G:2 * G],
                    op0=mybir.AluOpType.is_equal,
                    op1=mybir.AluOpType.mult,
                )
            wg.then_inc(wg_sem, 1)

            for t in range(NT - 1):
                if t >= BUFS:
                    # xs slot reuse: tile t-8's matmuls consumed it
                    vector.wait_ge(pe_sem, pe_count(t - BUFS))
                vector.wait_ge(dma_x[t % BUFS], xthresh(t))
                slot = t % BUFS
                vector.tensor_reduce(
                    out=xs_buf[:, slot * MV:(slot + 1) * MV],
                    in_=x_buf[:, slot * F + FC:(slot + 1) * F]
                    .rearrange("p (m j) -> p m j", j=HW),
                    axis=mybir.AxisListType.X,
                    op=mybir.AluOpType.add,
                ).then_inc(red_sem, 1)
            # tile 15 (slot 7): 3 piece reduces of 18 channels
            vector.wait_ge(pe_sem, pe_count(NT - 1 - BUFS))
            for p, (clo, chi) in enumerate(PIECE_CH):
                vector.wait_ge(dma_x[7], 32 + 16 * (p + 1))
                vector.tensor_reduce(
                    out=xs_buf[:, 7 * MV + (clo - MC):7 * MV + (chi - MC)],
                    in_=x_buf[:, 7 * F + clo * HW:7 * F + chi * HW]
                    .rearrange("p (m j) -> p m j", j=HW),
                    axis=mybir.AxisListType.X,
                    op=mybir.AluOpType.add,
                ).then_inc(red_sem, 1)
            # epilogue: j-reduce the raw-path PSUM into out[:, 0:MC]
            vector.wait_ge(pe_sem, pe_count(NT - 2) + 1)  # raw(15) done
            vector.tensor_reduce(
                out=out_sb[:, 0:MC],
                in_=psum_raw[:, :].rearrange("p (m j) -> p m j", j=HW),
                axis=mybir.AxisListType.X,
                op=mybir.AluOpType.add,
            ).then_inc(fin_sem, 1)

        # ---- TensorE: raw chunk + two xs matmuls per tile (fp32) ----
        @block.tensor
        def _(tensor):
            tensor.wait_ge(wg_sem, 1)
            for t in range(NT):
                slot = t % BUFS
                wt = w_sb[:, t * G:(t + 1) * G]
                first, last = (t == 0), (t == NT - 1)
                # raw chunk needs only the tile's first piece in DMA order
                tensor.wait_ge(dma_x[slot], xthresh(t))
                tensor.matmul(
                    out=psum_raw[:, :],
                    lhsT=wt,
                    rhs=x_buf[:, slot * F:slot * F + FC],
                    start=first,
                    stop=last,
                ).then_inc(pe_sem, 1)
                # xs halves; tile 15's reduces land as pieces 1-2 then 3
                tensor.wait_ge(red_sem, (t + 1) + (1 if last else 0))
                tensor.matmul(
                    out=psum_a[:, :],
                    lhsT=wt,
                    rhs=xs_buf[:, slot * MV:slot * MV + (MB - MC)],
                    start=first,
                    stop=last,
                ).then_inc(pe_sem, 1)
                if last:
                    tensor.wait_ge(red_sem, (t + 1) + 2)
                tensor.matmul(
                    out=psum_b[:, :],
                    lhsT=wt,
                    rhs=xs_buf[:, slot * MV + (MB - MC):(slot + 1) * MV],
                    start=first,
                    stop=last,
                ).then_inc(pe_sem, 1)

        # ---- ScalarE (ACT): PSUM -> SBUF copies ----
        @block.scalar
        def _(scalar):
            # warmup: the first activation op lazily DMAs the ACT function
            # table (~1.3 us); trigger it off the critical path
            scalar.activation(
                out=warm_sb[:, :],
                in_=warm_sb[:, :],
                func=mybir.ActivationFunctionType.Copy,
            )
            scalar.wait_ge(pe_sem, pe_count(NT - 1) - 1)  # mm_a(15)
            scalar.activation(
                out=out_sb[:, MC:MB],
                in_=psum_a[:, :],
                func=mybir.ActivationFunctionType.Copy,
            ).then_inc(fin_sem, 1)
            scalar.wait_ge(pe_sem, pe_count(NT - 1))      # mm_b(15)
            scalar.activation(
                out=out_sb[:, MB:ML],
                in_=psum_b[:, :],
                func=mybir.ActivationFunctionType.Copy,
            ).then_inc(fin_sem, 1)

    return nc


def _prepare(x, idx):
    x = np.asarray(x)
    if x.dtype != np.float32:
        x = x.astype(np.float32)
    idx = np.asarray(idx).astype(np.int64)
    counts = np.bincount(idx, minlength=G).astype(np.float64)
    scale = np.where(counts > 0, 1.0 / (counts * HW), 0.0).astype(np.float32)
    aux = np.zeros((P, G + G + NT), np.float32)
    aux[:, 0:G] = np.arange(G, dtype=np.float32)[None, :]
    aux[:, G:2 * G] = scale[None, :]
    aux[:, 2 * G:] = idx.reshape(NT, P).T.astype(np.float32)
    xr = x.reshape(N, M, HW)
    in_maps = []
    for k in range(CORES):
        shard = np.ascontiguousarray(xr[:, k * ML:(k + 1) * ML, :]).reshape(N, F)
        in_maps.append({"x": shard, "aux": aux})
    return in_maps


def run(x, tensor_list_assignmentindices, trace=False):
    in_maps = _prepare(x, tensor_list_assignmentindices)
    nc = _build()
    res = run_bass_kernel_spmd(nc, in_maps, core_ids=list(range(CORES)), trace=trace)
    outs = [np.asarray(r["out"]) for r in res.results]
    out = np.concatenate(outs, axis=1)  # [G, M]
    return out.reshape(G, M, 1, 1).astype(np.float32), res.exec_time_ns


def kernel(**inputs):
    out, _ = run(inputs["x"], inputs["tensor_list_assignmentindices"], trace=False)
    return out
